# revision 6
# baseline (speedup 1.0000x reference)
"""Causal single-head attention (B=4, N=2048, E=1024, D=64) on 8 TRN2 NeuronCores.

Sharding: core i handles batch b = i//2 with parity p = i%2.
 - Query rows: strided half (rows p, p+2, ...) -> causal workload is identical
   on every core, so one SPMD program serves all 8.
 - K/V loading: sequence-split — each core loads only keys [1024p, 1024p+1024),
   projects its half to kT/vT [64, 1024], then a pair AllGather (256 KB each)
   reconstructs the full projected kT/vT. This halves the dominant K/V DMA.

Per-core program (matmuls in fp32r, full PE rate):
  kTh = Wk.T @ KTh, qT = Wq.T @ QTh, vTh = Wv.T @ VTh   (PSUM over 8 E-chunks)
  AllGather(pair) kTh -> kT [64, 2048];  AllGather(pair) vTh -> vT
  vT -> PE-transpose -> v1 [2048, 65] (ones column = softmax denominator)
  per q-block j (256 local cols = 512 original rows), k-chunks c <= 4j+3:
    s^T  = kT_c.T @ qT_j      [128, 256]
    e    = exp(s^T / 8)       (ACT), causal mask via 0/1 multiply (DVE)
    po  += v1_c.T @ e         [65, 256]
  out_j = transpose(po)[:, :64] * recip(transpose(po)[:, 64])
"""
import os
import sys

sys.path.insert(0, "/opt/trn_rl_repo")

import numpy as np

B, N, E, D = 4, 2048, 1024, 64
NH = N // 2       # key half per core
NQL = N // 2      # local q rows per core
QB = 256          # local q-block width (in qT columns)
KC = 128          # k chunk
EC = 128          # E chunk
NEC = E // EC     # 8
PRECISION = os.environ.get("KERNEL_PRECISION", "f32r")
GROUPS = [[0, 1], [2, 3], [4, 5], [6, 7]]

_NC_CACHE = {}


def _build_nc():
    from concourse import bacc, mybir, tile
    from concourse.masks import make_identity

    f32 = mybir.dt.float32
    f32r = mybir.dt.float32r if PRECISION == "f32r" else mybir.dt.float32
    AF = mybir.ActivationFunctionType

    nc = bacc.Bacc()
    KTH = nc.dram_tensor("KTH", [E, NH], f32, kind="ExternalInput")
    QTH = nc.dram_tensor("QTH", [E, NQL], f32, kind="ExternalInput")
    VTH = nc.dram_tensor("VTH", [E, NH], f32, kind="ExternalInput")
    WK = nc.dram_tensor("WK", [E, D], f32, kind="ExternalInput")
    WQ = nc.dram_tensor("WQ", [E, D], f32, kind="ExternalInput")
    WV = nc.dram_tensor("WV", [E, D], f32, kind="ExternalInput")
    MASK = nc.dram_tensor("MASK", [4, KC, QB], f32, kind="ExternalInput")
    OUT = nc.dram_tensor("OUT", [NQL, D], f32, kind="ExternalOutput")

    with tile.TileContext(nc) as tc:
        with (
            tc.tile_pool(name="consts", bufs=1) as consts,
            tc.tile_pool(name="kin", bufs=3) as kin,
            tc.tile_pool(name="qin", bufs=3) as qin,
            tc.tile_pool(name="vin", bufs=3) as vin,
            tc.tile_pool(name="proj", bufs=1) as proj,
            tc.tile_pool(name="expp", bufs=24) as expp,
            tc.tile_pool(name="epi", bufs=2) as epi,
            tc.tile_pool(name="dram", bufs=1, space="DRAM") as dram,
        ):
            # ---- constants ----
            wk_all = consts.tile([EC, NEC, D], f32r, tag="wk")
            wq_all = consts.tile([EC, NEC, D], f32r, tag="wq")
            wv_all = consts.tile([EC, NEC, D], f32r, tag="wv")
            nc.sync.dma_start(wk_all[:], WK.rearrange("(c p) m -> p c m", p=EC).bitcast(f32r))
            nc.sync.dma_start(wq_all[:], WQ.rearrange("(c p) m -> p c m", p=EC).bitcast(f32r))
            nc.sync.dma_start(wv_all[:], WV.rearrange("(c p) m -> p c m", p=EC).bitcast(f32r))
            masks = consts.tile([KC, 4, QB], f32r, tag="mask")
            nc.sync.dma_start(masks[:], MASK.rearrange("m p q -> p m q").bitcast(f32r))
            ident = consts.tile([128, 128], f32, tag="ident")
            make_identity(nc, ident[:])

            kT_sb = proj.tile([D, 2, NH], f32r, tag="kT")
            qT_sb = proj.tile([D, NQL], f32r, tag="qT")
            kh_sb = proj.tile([D, NH], f32, tag="kh")
            vh_sb = proj.tile([D, NH], f32, tag="vh")
            vT_sb = proj.tile([D, 2, NH], f32, tag="vT")
            v1_sb = proj.tile([KC, N // KC, D + 1], f32r, tag="v1")

            kh_dram = dram.tile([D, NH], f32, name="kh_dram")
            kf_dram = dram.tile([2, D, NH], f32, name="kf_dram")
            vh_dram = dram.tile([D, NH], f32, name="vh_dram")
            vf_dram = dram.tile([2, D, NH], f32, name="vf_dram")

            # ---- k/q projections over own halves ----
            with tc.tile_pool(name="psKQ", bufs=1, space="PSUM") as psKQ:
                pk = [psKQ.tile([D, 512], f32, tag=f"pk{t}", name=f"pk{t}") for t in range(2)]
                pq = [psKQ.tile([D, 512], f32, tag=f"pq{t}", name=f"pq{t}") for t in range(2)]
                for c in range(NEC):
                    kt = kin.tile([EC, NH], f32r, tag="kt")
                    nc.sync.dma_start(kt[:], KTH[EC * c:EC * (c + 1), :].bitcast(f32r))
                    qt = qin.tile([EC, NQL], f32r, tag="qt")
                    nc.sync.dma_start(qt[:], QTH[EC * c:EC * (c + 1), :].bitcast(f32r))
                    for t in range(2):
                        nc.tensor.matmul(pk[t][:], wk_all[:, c, :], kt[:, 512 * t:512 * (t + 1)],
                                         start=(c == 0), stop=(c == NEC - 1))
                    for t in range(2):
                        nc.tensor.matmul(pq[t][:], wq_all[:, c, :], qt[:, 512 * t:512 * (t + 1)],
                                         start=(c == 0), stop=(c == NEC - 1))
                for t in range(2):
                    nc.scalar.copy(kh_sb[:, 512 * t:512 * (t + 1)], pk[t][:])
                for t in range(2):
                    nc.scalar.copy(qT_sb[:, 512 * t:512 * (t + 1)], pq[t][:])

            # k exchange: pair AllGather of projected half
            nc.sync.dma_start(kh_dram[:], kh_sb[:])
            nc.gpsimd.collective_compute(
                "AllGather", nc_alu_bypass(), replica_groups=GROUPS,
                ins=[kh_dram[:]], outs=[kf_dram[:]],
            )
            nc.sync.dma_start(kT_sb[:], kf_dram.rearrange("r p n -> p r n").bitcast(f32r))

            # ---- v projection over own half + exchange + transpose ----
            nc.gpsimd.memset(v1_sb[:].bitcast(f32), 1.0)
            with tc.tile_pool(name="psV", bufs=1, space="PSUM") as psV:
                pv = [psV.tile([D, 512], f32, tag=f"pv{t}", name=f"pv{t}") for t in range(2)]
                for c in range(NEC):
                    vt = vin.tile([EC, NH], f32r, tag="vt")
                    nc.sync.dma_start(vt[:], VTH[EC * c:EC * (c + 1), :].bitcast(f32r))
                    for t in range(2):
                        nc.tensor.matmul(pv[t][:], wv_all[:, c, :], vt[:, 512 * t:512 * (t + 1)],
                                         start=(c == 0), stop=(c == NEC - 1))
                for t in range(2):
                    nc.scalar.copy(vh_sb[:, 512 * t:512 * (t + 1)], pv[t][:])
            nc.sync.dma_start(vh_dram[:], vh_sb[:])
            nc.gpsimd.collective_compute(
                "AllGather", nc_alu_bypass(), replica_groups=GROUPS,
                ins=[vh_dram[:]], outs=[vf_dram[:]],
            )
            nc.sync.dma_start(vT_sb[:], vf_dram.rearrange("r p n -> p r n"))

            with tc.tile_pool(name="psVT", bufs=2, space="PSUM") as psVT:
                for t in range(N // KC):
                    pvt = psVT.tile([KC, D], f32, tag="pvt")
                    nc.tensor.transpose(pvt[:], vT_sb[:, t // 8, KC * (t % 8):KC * (t % 8 + 1)], ident[0:D, 0:D])
                    nc.scalar.copy(v1_sb[:, t, 0:D], pvt[:])

            # ---- attention ----
            with tc.tile_pool(name="psE", bufs=1, space="PSUM") as psE:
                for j in range(4):
                    po = psE.tile([D + 1, QB], f32, tag="po")
                    nch = 4 * j + 4
                    for c in range(nch):
                        ps = psE.tile([KC, QB], f32, tag="ps")
                        nc.tensor.matmul(ps[:], kT_sb[:, c // 8, KC * (c % 8):KC * (c % 8 + 1)],
                                         qT_sb[:, QB * j:QB * (j + 1)], start=True, stop=True)
                        ex = expp.tile([KC, QB], f32r, tag="ex")
                        nc.scalar.activation(ex[:], ps[:], AF.Exp, scale=0.125)
                        if c >= 4 * j:
                            nc.vector.tensor_mul(ex[:], ex[:], masks[:, c - 4 * j, :])
                        nc.tensor.matmul(po[:], v1_sb[:, c, :], ex[:],
                                         start=(c == 0), stop=(c == nch - 1))
                    pot = epi.tile([D + 1, QB], f32, tag="pot")
                    nc.scalar.copy(pot[:], po[:])
                    ob = epi.tile([KC, 2, D], f32, tag="ob")
                    for h in range(2):
                        pq2 = psE.tile([KC, D + 1], f32, tag="pq2")
                        nc.tensor.transpose(pq2[:], pot[:, KC * h:KC * (h + 1)],
                                            ident[0:D + 1, 0:D + 1])
                        rcp = epi.tile([KC, 1], f32, tag="rcp")
                        nc.vector.reciprocal(rcp[:], pq2[:, D:D + 1])
                        nc.vector.tensor_scalar_mul(ob[:, h, :], pq2[:, 0:D], rcp[:])
                    nc.sync.dma_start(
                        OUT[QB * j:QB * (j + 1), :].rearrange("(h p) d -> p h d", p=KC),
                        ob[:])

    nc.finalize()
    return nc


def nc_alu_bypass():
    from concourse import mybir
    return mybir.AluOpType.bypass


def get_nc():
    if "nc" not in _NC_CACHE:
        _NC_CACHE["nc"] = _build_nc()
    return _NC_CACHE["nc"]


def shard_inputs(K, Q, V, Wk, Wq, Wv):
    K, Q, V = np.asarray(K), np.asarray(Q), np.asarray(V)
    Wk, Wq, Wv = (np.ascontiguousarray(np.asarray(a), dtype=np.float32) for a in (Wk, Wq, Wv))
    kk = np.arange(KC)
    qq = np.arange(QB)
    masks = {}
    for p in range(2):
        masks[p] = np.stack([
            (kk[:, None] + KC * m <= 2 * qq[None, :] + p).astype(np.float32)
            for m in range(4)
        ])
    in_maps = []
    for core in range(8):
        b, p = core // 2, core % 2
        in_maps.append({
            "KTH": np.ascontiguousarray(K[b].T[:, NH * p:NH * (p + 1)], dtype=np.float32),
            "QTH": np.ascontiguousarray(Q[b].T[:, p::2], dtype=np.float32),
            "VTH": np.ascontiguousarray(V[b].T[:, NH * p:NH * (p + 1)], dtype=np.float32),
            "WK": Wk, "WQ": Wq, "WV": Wv,
            "MASK": masks[p],
        })
    return in_maps


def gather_outputs(outs):
    full = np.zeros((B, N, D), np.float32)
    for core in range(8):
        b, p = core // 2, core % 2
        full[b, p::2] = outs[core]
    return full


def kernel(K, Q, V, Wk, Wq, Wv):
    from concourse.bass_utils import run_bass_kernel_spmd

    in_maps = shard_inputs(K, Q, V, Wk, Wq, Wv)
    nc = get_nc()
    res = run_bass_kernel_spmd(nc, in_maps, list(range(8)))
    return gather_outputs([res.results[i]["OUT"] for i in range(8)])


# revision 13
# speedup vs baseline: 1.6356x; 1.6356x over previous
"""Causal single-head attention (B=4, N=2048, E=1024, D=64) on 8 TRN2 NeuronCores.

Sharding: core i handles batch b = i//2, query rows with parity p = i%2
(rows p, p+2, ...). The row-interleaved split makes the causal workload
identical on every core, so one SPMD program serves all 8. K/V are loaded in
full per core (no collectives); Q is the strided half.

The kernel streams K/V in 4 strips of 512 keys so that projections, scores,
softmax and AV all pipeline behind the DMA stream (fp32r matmuls, full PE
rate):
  prologue: qT = Wq.T @ Q.T  [64, 1024]  (4 blocks of 256)
  per strip s (keys [512s, 512s+512)):
    kT_s = Wk.T @ KT_s, vT_s = Wv.T @ VT_s    (PSUM over 8 E-chunks)
    vT_s -> PE-transpose -> v1 rows [512s..] ([k, 65], ones column)
    for q-block j >= s, chunk c in 4s..4s+3:
      e = exp((kT_c.T @ qT_j) / 8) [* causal 0/1 mask if j == s]
      po[j] += v1_c.T @ e          [65, 256] (row 64 = softmax denominator)
    epilogue for block s (po[s] complete):
      out_s = transpose(po[s])[:, :64] * recip(transpose(po[s])[:, 64])
"""
import os
import sys

sys.path.insert(0, "/opt/trn_rl_repo")

import numpy as np

B, N, E, D = 4, 2048, 1024, 64
NQL = N // 2      # local q rows per core
QB = 256          # local q-block width (in qT columns)
KC = 128          # k chunk
EC = 128          # E chunk
NEC = E // EC     # 8
SW = 256          # strip width (keys per strip)
NS = N // SW      # 8 strips
PRECISION = os.environ.get("KERNEL_PRECISION", "f32r")

_NC_CACHE = {}


def _build_nc(reps=1):
    from concourse import bacc, mybir, tile
    from concourse.masks import make_identity

    f32 = mybir.dt.float32
    f32r = mybir.dt.float32r if PRECISION == "f32r" else mybir.dt.float32
    AF = mybir.ActivationFunctionType

    nc = bacc.Bacc()
    KT = nc.dram_tensor("KT", [NS, EC, NEC, SW], f32, kind="ExternalInput")
    QT = nc.dram_tensor("QT", [EC, NEC, NQL], f32, kind="ExternalInput")
    VT = nc.dram_tensor("VT", [NS, EC, NEC, SW], f32, kind="ExternalInput")
    WK = nc.dram_tensor("WK", [EC, NEC, D], f32, kind="ExternalInput")
    WQ = nc.dram_tensor("WQ", [EC, NEC, D], f32, kind="ExternalInput")
    WV = nc.dram_tensor("WV", [EC, NEC, D], f32, kind="ExternalInput")
    MASK = nc.dram_tensor("MASK", [KC, 4, QB], mybir.dt.bfloat16, kind="ExternalInput")
    OUT = nc.dram_tensor("OUT", [NQL, D], f32, kind="ExternalOutput")

    with tile.TileContext(nc) as tc:
        for _rep in range(reps):
            with (
                tc.tile_pool(name=f"consts{_rep}", bufs=1) as consts,
                tc.tile_pool(name=f"qin{_rep}", bufs=2) as qin,
                tc.tile_pool(name=f"kin{_rep}", bufs=2) as kin,
                tc.tile_pool(name=f"vin{_rep}", bufs=2) as vin,
                tc.tile_pool(name=f"proj{_rep}", bufs=1) as proj,
                tc.tile_pool(name=f"expp{_rep}", bufs=6) as expp,
                tc.tile_pool(name=f"epi{_rep}", bufs=2) as epi,
                tc.tile_pool(name=f"psA{_rep}", bufs=1, space="PSUM") as psA,
            ):
                # ---- constants ----
                wk_all = consts.tile([EC, NEC, D], f32r, tag="wk")
                wq_all = consts.tile([EC, NEC, D], f32r, tag="wq")
                wv_all = consts.tile([EC, NEC, D], f32r, tag="wv")
                nc.sync.dma_start(wq_all[:], WQ[:].bitcast(f32r))
                nc.sync.dma_start(wk_all[:], WK[:].bitcast(f32r))
                nc.sync.dma_start(wv_all[:], WV[:].bitcast(f32r))
                masks = consts.tile([KC, 4, QB], mybir.dt.bfloat16, tag="mask")
                nc.sync.dma_start(masks[:], MASK[:])
                ident = consts.tile([128, 128], f32, tag="ident")
                make_identity(nc, ident[:])

                kT_sb = proj.tile([D, N], f32r, tag="kT")
                qT_sb = proj.tile([D, NQL], f32r, tag="qT")
                v1_sb = proj.tile([KC, N // KC, D + 1], f32r, tag="v1")
                nc.gpsimd.memset(v1_sb[:].bitcast(f32), 1.0)

                # PSUM pools: po[0..3] (4 banks) + ps/pq2 (2) + pkq (1) + pv/pvt (1)
                po = [psA.tile([D + 1, QB], f32, tag=f"po{j}", name=f"po{j}", bufs=1)
                      for j in range(4)]

                # ---- prologue: q projections (4 blocks of 256) ----
                qt = qin.tile([EC, NEC, NQL], f32r, tag="qt", bufs=1)
                nc.sync.dma_start(qt[:], QT[:].bitcast(f32r))
                for j in range(4):
                    pkq = psA.tile([D, SW], f32, tag="pkq", name="pkq", bufs=1)
                    for c in range(NEC):
                        nc.tensor.matmul(pkq[:, 0:QB], wq_all[:, c, :],
                                         qt[:, c, QB * j:QB * (j + 1)],
                                         start=(c == 0), stop=(c == NEC - 1))
                    nc.vector.tensor_copy(qT_sb[:, QB * j:QB * (j + 1)], pkq[:, 0:QB])

                # ---- strips ----
                for s in range(NS):
                    # k projection for keys [SW*s, SW*(s+1))
                    kt = kin.tile([EC, NEC, SW], f32r, tag="kt")
                    nc.sync.dma_start(kt[:], KT[s].bitcast(f32r))
                    pkq = psA.tile([D, SW], f32, tag="pkq", name="pkq", bufs=1)
                    for c in range(NEC):
                        nc.tensor.matmul(pkq[:], wk_all[:, c, :], kt[:, c, :],
                                         start=(c == 0), stop=(c == NEC - 1))
                    nc.vector.tensor_copy(kT_sb[:, SW * s:SW * (s + 1)], pkq[:])

                    # v projection + transpose into v1
                    vt = vin.tile([EC, NEC, SW], f32r, tag="vt")
                    nc.sync.dma_start(vt[:], VT[s].bitcast(f32r))
                    pv = psA.tile([D, SW], f32, tag="pv", name="pv", bufs=1)
                    for c in range(NEC):
                        nc.tensor.matmul(pv[:], wv_all[:, c, :], vt[:, c, :],
                                         start=(c == 0), stop=(c == NEC - 1))
                    vT_st = proj.tile([D, SW], f32, tag="vT")
                    nc.vector.tensor_copy(vT_st[:], pv[:])
                    for t in range(SW // KC):
                        pvt = psA.tile([KC, D], f32, tag="ps", name="pvt", bufs=2)
                        nc.tensor.transpose(pvt[:], vT_st[:, KC * t:KC * (t + 1)], ident[0:D, 0:D])
                        nc.vector.tensor_copy(v1_sb[:, (SW // KC) * s + t, 0:D], pvt[:])

                    # attention for the new k chunks against q-blocks j >= s//2
                    for j in range((SW * s) // (2 * QB), 4):
                        for m in range(SW // KC):
                            c = (SW // KC) * s + m
                            ps = psA.tile([KC, QB], f32, tag="ps", name="ps", bufs=2)
                            nc.tensor.matmul(ps[:], kT_sb[:, KC * c:KC * (c + 1)],
                                             qT_sb[:, QB * j:QB * (j + 1)], start=True, stop=True)
                            ex = expp.tile([KC, QB], f32r, tag="ex")
                            nc.scalar.activation(ex[:], ps[:], AF.Exp, scale=0.125)
                            if c >= 4 * j:
                                nc.vector.tensor_mul(ex[:], ex[:], masks[:, c - 4 * j, :])
                            nc.tensor.matmul(po[j][:], v1_sb[:, c, :], ex[:],
                                             start=(c == 0), stop=(c == 4 * j + 3))

                    # epilogue when a q-block just completed (strip covered its last chunks)
                    if (SW * (s + 1)) % (2 * QB) == 0:
                        jj = (SW * (s + 1)) // (2 * QB) - 1
                        pot = epi.tile([D + 1, QB], f32, tag="pot")
                        nc.vector.tensor_copy(pot[:], po[jj][:])
                        ob = epi.tile([KC, 2, D], f32, tag="ob")
                        for h in range(2):
                            pq2 = psA.tile([KC, D + 1], f32, tag="ps", name="pq2", bufs=2)
                            nc.tensor.transpose(pq2[:], pot[:, KC * h:KC * (h + 1)],
                                                ident[0:D + 1, 0:D + 1])
                            rcp = epi.tile([KC, 1], f32, tag="rcp")
                            nc.vector.reciprocal(rcp[:], pq2[:, D:D + 1])
                            nc.vector.tensor_scalar_mul(ob[:, h, :], pq2[:, 0:D], rcp[:])
                        nc.sync.dma_start(
                            OUT[QB * jj:QB * (jj + 1), :].rearrange("(h p) d -> p h d", p=KC),
                            ob[:])

    nc.finalize()
    return nc


def get_nc(reps=1):
    key = ("nc", reps)
    if key not in _NC_CACHE:
        _NC_CACHE[key] = _build_nc(reps)
    return _NC_CACHE[key]


def shard_inputs(K, Q, V, Wk, Wq, Wv):
    K, Q, V = np.asarray(K), np.asarray(Q), np.asarray(V)
    Wkx, Wqx, Wvx = (
        np.ascontiguousarray(np.asarray(a, dtype=np.float32).reshape(NEC, EC, D).transpose(1, 0, 2))
        for a in (Wk, Wq, Wv))
    import ml_dtypes
    kk = np.arange(KC)
    qq = np.arange(QB)
    masks = {}
    for p in range(2):
        m4 = np.stack([
            (kk[:, None] + KC * m <= 2 * qq[None, :] + p).astype(np.float32)
            for m in range(4)
        ])  # [4, 128, 256]
        masks[p] = np.ascontiguousarray(m4.transpose(1, 0, 2).astype(ml_dtypes.bfloat16))
    in_maps = []
    for core in range(8):
        b, p = core // 2, core % 2
        kx = np.ascontiguousarray(
            K[b].T.reshape(NEC, EC, NS, SW).transpose(2, 1, 0, 3), dtype=np.float32)
        vx = np.ascontiguousarray(
            V[b].T.reshape(NEC, EC, NS, SW).transpose(2, 1, 0, 3), dtype=np.float32)
        qx = np.ascontiguousarray(
            Q[b].T[:, p::2].reshape(NEC, EC, NQL).transpose(1, 0, 2), dtype=np.float32)
        in_maps.append({
            "KT": kx,
            "QT": qx,
            "VT": vx,
            "WK": Wkx, "WQ": Wqx, "WV": Wvx,
            "MASK": masks[p],
        })
    return in_maps


def gather_outputs(outs):
    full = np.zeros((B, N, D), np.float32)
    for core in range(8):
        b, p = core // 2, core % 2
        full[b, p::2] = outs[core]
    return full


def kernel(K, Q, V, Wk, Wq, Wv):
    from concourse.bass_utils import run_bass_kernel_spmd

    in_maps = shard_inputs(K, Q, V, Wk, Wq, Wv)
    nc = get_nc()
    res = run_bass_kernel_spmd(nc, in_maps, list(range(8)))
    return gather_outputs([res.results[i]["OUT"] for i in range(8)])
